# revision 6
# baseline (speedup 1.0000x reference)
"""Single-head attention on 8 TRN2 NeuronCores.

Sharding: data-parallel over batch (4) x sequence-parallel over the query
dim (2 halves of 2048) = 8 independent shards, no collectives. Each core
receives its batch's hidden_state pre-transposed ([E, S] bf16) with the
local query block rotated to columns 0:2048 (key order is irrelevant to
softmax+AV as long as K and V share it).

Per-core kernel (all matmuls bf16, PSUM f32):
  hsT [1024, 4096]
  KV^T = [Wk|Wv]^T @ hsT      -> kvT [128, 4096]  (rows 0:64 K^T, 64:128 V^T)
  Q^T  = (Wq*s)^T @ hsT[:, :2048] -> qT [64, 2048]   (s = 1/sqrt(64) folded in)
  V    = transpose(V^T) chunks -> vaug [128, 65] per k-chunk (col 64 = ones)
  S^T[k,q] = K^T.T @ Q^T  (per 128-k-chunk, 512-q-block)
  P^T = exp(S^T)            (scores bounded ~|2|, no max-subtraction needed)
  outT[65, q] = sum_k vaug^T @ P^T   (row 64 = softmax denominator)
  out[q, 64] = transpose(outT) * recip(denominator)
"""

import numpy as np
import ml_dtypes
from contextlib import ExitStack

import concourse.bass as bass
import concourse.bacc as bacc
import concourse.mybir as mybir
import concourse.tile as tile
from concourse.bass_utils import run_bass_kernel_spmd
from concourse.masks import make_identity

B, S, E, H = 4, 4096, 1024, 64
M = 2048            # queries per core
NCORES = 8
EC = E // 128       # 8 contraction chunks
ST = S // 512       # 8 projection column tiles
KC = S // 128       # 32 key chunks
QB = M // 512       # 4 query blocks
G = 2               # key chunks per exp group (psum banks per S^T tile)
bf16 = mybir.dt.bfloat16
f32 = mybir.dt.float32

_nc_cache = None
last_results = None  # BassKernelResults from the most recent run (for test.py)


def _build():
    nc = bacc.Bacc("TRN2", target_bir_lowering=False, debug=False,
                   num_devices=NCORES)
    hsT = nc.dram_tensor("hsT", [E, S], bf16, kind="ExternalInput")
    wkv = nc.dram_tensor("wkv", [E, 128], bf16, kind="ExternalInput")
    wq = nc.dram_tensor("wq", [E, H], bf16, kind="ExternalInput")
    bkv = nc.dram_tensor("bkv", [128, 1], f32, kind="ExternalInput")
    bqt = nc.dram_tensor("bq", [H, 1], f32, kind="ExternalInput")
    out = nc.dram_tensor("out", [M, H], f32, kind="ExternalOutput")

    with tile.TileContext(nc) as tc:
        with ExitStack() as ctx:
            consts = ctx.enter_context(tc.tile_pool(name="consts", bufs=1))
            sb = ctx.enter_context(tc.tile_pool(name="sb", bufs=1))
            pt_pool = ctx.enter_context(tc.tile_pool(name="pt", bufs=3))
            oT_pool = ctx.enter_context(tc.tile_pool(name="oT", bufs=2))
            out_pool = ctx.enter_context(tc.tile_pool(name="osb", bufs=3))
            psum_proj = ctx.enter_context(
                tc.tile_pool(name="pp", bufs=2, space=bass.MemorySpace.PSUM))
            psum_st = ctx.enter_context(
                tc.tile_pool(name="pst", bufs=2, space=bass.MemorySpace.PSUM))
            psum_av = ctx.enter_context(
                tc.tile_pool(name="pav", bufs=2, space=bass.MemorySpace.PSUM))

            wkv_sb = consts.tile([128, EC, 128], bf16)
            nc.sync.dma_start(wkv_sb[:], wkv.ap().rearrange("(c p) h -> p c h", p=128))
            wq_sb = consts.tile([128, EC, H], bf16)
            nc.sync.dma_start(wq_sb[:], wq.ap().rearrange("(c p) h -> p c h", p=128))
            bkv_sb = consts.tile([128, 1], f32)
            nc.sync.dma_start(bkv_sb[:], bkv.ap())
            bq_sb = consts.tile([H, 1], f32)
            nc.sync.dma_start(bq_sb[:], bqt.ap())
            ident_bf = consts.tile([128, 128], bf16)
            make_identity(nc, ident_bf[:])
            ident_f32 = consts.tile([128, 128], f32)
            make_identity(nc, ident_f32[:])

            # input: one tile per 512-column block, all 8 e-chunks
            hsT_r = hsT.ap().rearrange("(c p) s -> p c s", p=128)
            hs_t = []
            for j in range(ST):
                t = sb.tile([128, EC, 512], bf16, tag=f"hs{j}", name=f"hs{j}")
                nc.sync.dma_start(t[:], hsT_r[:, :, bass.ts(j, 512)])
                hs_t.append(t)

            kv_t = [sb.tile([128, 512], bf16, tag=f"kv{t}", name=f"kv{t}") for t in range(ST)]
            q_t = [sb.tile([H, 512], bf16, tag=f"q{t}", name=f"q{t}") for t in range(QB)]
            # copies of K^T / Q^T at partition offset 64 so score matmul
            # pairs can row-tile the PE array (rows 0:63 and 64:127 run
            # concurrently: the contraction dim is only h=64)
            kd_t = [sb.tile([128, 512], bf16, tag=f"kd{t}", name=f"kd{t}") for t in range(ST)]
            qd_t = [sb.tile([128, 512], bf16, tag=f"qd{t}", name=f"qd{t}") for t in range(QB)]
            vaug = [sb.tile([128, H + 1], bf16, tag=f"v{kc}", name=f"v{kc}") for kc in range(KC)]
            for kc in range(KC):
                nc.gpsimd.memset(vaug[kc][:, H:H + 1], 1.0)

            # projections + V transposes
            for t in range(ST):
                pkv = psum_proj.tile([128, 512], f32, tag="proj")
                for c in range(EC):
                    nc.tensor.matmul(pkv[:], wkv_sb[:, c, :], hs_t[t][:, c, :],
                                     start=(c == 0), stop=(c == EC - 1))
                nc.vector.tensor_scalar_add(kv_t[t][:], pkv[:], bkv_sb[:])
                nc.sync.dma_start(kd_t[t][64:128, :], kv_t[t][0:64, :])
                if t < QB:
                    pq = psum_proj.tile([H, 512], f32, tag="proj")
                    for c in range(EC):
                        nc.tensor.matmul(pq[:], wq_sb[:, c, :], hs_t[t][:, c, :],
                                         start=(c == 0), stop=(c == EC - 1))
                    nc.vector.tensor_scalar_add(q_t[t][:], pq[:], bq_sb[:])
                    nc.sync.dma_start(qd_t[t][64:128, :], q_t[t][:])
                for kk in range(4):
                    kc = 4 * t + kk
                    ptr = psum_proj.tile([128, H], bf16, tag="proj")
                    nc.tensor.transpose(ptr[:], kv_t[t][64:128, bass.ts(kk, 128)],
                                        ident_bf[64:128, 64:128])
                    nc.vector.tensor_copy(vaug[kc][:, 0:H], ptr[:])

            # main loop: scores^T -> exp -> AV accumulate
            for qb in range(QB):
                pav = psum_av.tile([H + 1, 512], f32, tag="av")
                for g in range(KC // G):
                    pst = psum_st.tile([128, G, 512], f32, tag="st")
                    for i in range(G):
                        kc = G * g + i
                        if i % 2 == 0:
                            nc.tensor.matmul(pst[:, i, :],
                                             kv_t[kc // 4][0:64, bass.ts(kc % 4, 128)],
                                             q_t[qb][:],
                                             start=True, stop=True)
                        else:
                            nc.tensor.matmul(pst[:, i, :],
                                             kd_t[kc // 4][64:128, bass.ts(kc % 4, 128)],
                                             qd_t[qb][64:128, :],
                                             start=True, stop=True)
                    ptile = pt_pool.tile([128, G, 512], bf16, tag="pt")
                    nc.scalar.activation(ptile[:], pst[:],
                                         mybir.ActivationFunctionType.Exp)
                    for i in range(G):
                        kc = G * g + i
                        nc.tensor.matmul(pav[:], vaug[kc][:], ptile[:, i, :],
                                         start=(kc == 0), stop=(kc == KC - 1))
                # epilogue: normalize + transpose + store
                oT = oT_pool.tile([H + 1, 512], f32, tag="oT")
                nc.vector.tensor_copy(oT[:], pav[:])
                nc.vector.reciprocal(oT[H:H + 1, :], oT[H:H + 1, :])
                for j in range(4):
                    ptr2 = psum_proj.tile([128, H + 1], f32, tag="proj")
                    nc.tensor.transpose(ptr2[:], oT[:, bass.ts(j, 128)],
                                        ident_f32[0:H + 1, 0:H + 1])
                    rec = out_pool.tile([128, 1], f32, tag="rec")
                    nc.vector.tensor_copy(rec[:], ptr2[:, H:H + 1])
                    osb = out_pool.tile([128, H], f32, tag="osb")
                    nc.vector.tensor_scalar_mul(osb[:], ptr2[:, 0:H], rec[:])
                    nc.sync.dma_start(
                        out.ap()[512 * qb + 128 * j:512 * qb + 128 * (j + 1), :],
                        osb[:])

    nc.compile()
    return nc


def _get_nc():
    global _nc_cache
    if _nc_cache is None:
        _nc_cache = _build()
    return _nc_cache


def kernel(hidden_state, Wq, bq, Wk, bk, Wv, bv):
    global last_results
    nc = _get_nc()
    scale = 1.0 / np.sqrt(np.float32(H))
    wkv_np = np.ascontiguousarray(
        np.concatenate([Wk, Wv], axis=1)).astype(ml_dtypes.bfloat16)
    wq_np = np.ascontiguousarray(Wq * scale).astype(ml_dtypes.bfloat16)
    bkv_np = np.ascontiguousarray(
        np.concatenate([bk, bv]).reshape(128, 1)).astype(np.float32)
    bq_np = np.ascontiguousarray((bq * scale).reshape(H, 1)).astype(np.float32)

    in_maps = []
    for core in range(NCORES):
        b, qh = core // 2, core % 2
        hs = np.asarray(hidden_state[b], dtype=np.float32)
        if qh:
            hs = np.concatenate([hs[M:], hs[:M]], axis=0)
        hsT_np = np.ascontiguousarray(hs.T).astype(ml_dtypes.bfloat16)
        in_maps.append({"hsT": hsT_np, "wkv": wkv_np, "wq": wq_np,
                        "bkv": bkv_np, "bq": bq_np})

    last_results = run_bass_kernel_spmd(nc, in_maps, core_ids=list(range(NCORES)))
    out = np.empty((B, S, H), np.float32)
    for core in range(NCORES):
        b, qh = core // 2, core % 2
        out[b, M * qh:M * (qh + 1)] = last_results.results[core]["out"]
    return out


# revision 11
# speedup vs baseline: 1.1602x; 1.1602x over previous
"""Single-head attention on 8 TRN2 NeuronCores.

Sharding: data-parallel over batch (4) x sequence-parallel over the query
dim (2 halves of 2048) = 8 independent shards, no collectives. Each core
receives its batch's hidden_state pre-transposed ([E, S] bf16) with the
local query block rotated to columns 0:2048 (key order is irrelevant to
softmax+AV as long as K and V share it).

Per-core kernel (all matmuls bf16, PSUM f32):
  hsT [1024, 4096]
  KV^T = [Wk|Wv]^T @ hsT      -> kvT [128, 4096]  (rows 0:64 K^T, 64:128 V^T)
  Q^T  = (Wq*s)^T @ hsT[:, :2048] -> qT [64, 2048]   (s = 1/sqrt(64) folded in)
  V    = transpose(V^T) chunks -> vaug [128, 65] per k-chunk (col 64 = ones)
  S^T[k,q] = K^T.T @ Q^T  (per 128-k-chunk, 512-q-block)
  P^T = exp(S^T)            (scores bounded ~|2|, no max-subtraction needed)
  outT[65, q] = sum_k vaug^T @ P^T   (row 64 = softmax denominator)
  out[q, 64] = transpose(outT) * recip(denominator)
"""

import numpy as np
import ml_dtypes
from contextlib import ExitStack

import concourse.bass as bass
import concourse.bacc as bacc
import concourse.mybir as mybir
import concourse.tile as tile
from concourse.bass_utils import run_bass_kernel_spmd
from concourse.masks import make_identity

B, S, E, H = 4, 4096, 1024, 64
M = 2048            # queries per core
NCORES = 8
EC = E // 128       # 8 contraction chunks
ST = S // 512       # 8 projection column tiles
KC = S // 128       # 32 key chunks
QB = M // 512       # 4 query blocks
G = 2               # key chunks per exp group (psum banks per S^T tile)
bf16 = mybir.dt.bfloat16
f32 = mybir.dt.float32

_nc_cache = None
last_results = None  # BassKernelResults from the most recent run (for test.py)


def _build():
    nc = bacc.Bacc("TRN2", target_bir_lowering=False, debug=False,
                   num_devices=NCORES)
    # hsb[j, p, c, s] = hsT[c*128+p, 512j+s] — every DMA fully contiguous
    hsb = nc.dram_tensor("hsb", [ST, 128, EC, 512], bf16, kind="ExternalInput")
    wkv = nc.dram_tensor("wkv", [128, EC, 128], bf16, kind="ExternalInput")
    wq = nc.dram_tensor("wq", [128, EC, H], bf16, kind="ExternalInput")
    bkv = nc.dram_tensor("bkv", [128, 1], f32, kind="ExternalInput")
    bqt = nc.dram_tensor("bq", [H, 1], f32, kind="ExternalInput")
    out = nc.dram_tensor("out", [M, H], f32, kind="ExternalOutput")

    with tile.TileContext(nc) as tc:
        with ExitStack() as ctx:
            consts = ctx.enter_context(tc.tile_pool(name="consts", bufs=1))
            sb = ctx.enter_context(tc.tile_pool(name="sb", bufs=1))
            pt_pool = ctx.enter_context(tc.tile_pool(name="pt", bufs=3))
            oT_pool = ctx.enter_context(tc.tile_pool(name="oT", bufs=2))
            out_pool = ctx.enter_context(tc.tile_pool(name="osb", bufs=3))
            psum_proj = ctx.enter_context(
                tc.tile_pool(name="pp", bufs=2, space=bass.MemorySpace.PSUM))
            psum_st = ctx.enter_context(
                tc.tile_pool(name="pst", bufs=2, space=bass.MemorySpace.PSUM))
            psum_av = ctx.enter_context(
                tc.tile_pool(name="pav", bufs=2, space=bass.MemorySpace.PSUM))

            wkv_sb = consts.tile([128, EC, 128], bf16)
            nc.sync.dma_start(wkv_sb[:], wkv.ap())
            wq_sb = consts.tile([128, EC, H], bf16)
            nc.sync.dma_start(wq_sb[:], wq.ap())
            bkv_sb = consts.tile([128, 1], f32)
            nc.sync.dma_start(bkv_sb[:], bkv.ap())
            bq_sb = consts.tile([H, 1], f32)
            nc.sync.dma_start(bq_sb[:], bqt.ap())
            ident_bf = consts.tile([128, 128], bf16)
            make_identity(nc, ident_bf[:])
            ident_f32 = consts.tile([128, 128], f32)
            make_identity(nc, ident_f32[:])

            # input: one tile per 512-column block, all 8 e-chunks
            hs_t = []
            for j in range(ST):
                t = sb.tile([128, EC, 512], bf16, tag=f"hs{j}", name=f"hs{j}")
                nc.sync.dma_start(t[:], hsb.ap()[j])
                hs_t.append(t)

            kv_t = [sb.tile([128, 512], bf16, tag=f"kv{t}", name=f"kv{t}") for t in range(ST)]
            q_t = [sb.tile([H, 512], bf16, tag=f"q{t}", name=f"q{t}") for t in range(QB)]
            # copies of K^T / Q^T at partition offset 64 so score matmul
            # pairs can row-tile the PE array (rows 0:63 and 64:127 run
            # concurrently: the contraction dim is only h=64)
            kd_t = [sb.tile([128, 512], bf16, tag=f"kd{t}", name=f"kd{t}") for t in range(ST)]
            qd_t = [sb.tile([128, 512], bf16, tag=f"qd{t}", name=f"qd{t}") for t in range(QB)]
            vaug = [sb.tile([128, H + 1], bf16, tag=f"v{kc}", name=f"v{kc}") for kc in range(KC)]
            for kc in range(KC):
                nc.gpsimd.memset(vaug[kc][:, H:H + 1], 1.0)

            # projections + V transposes
            for t in range(ST):
                pkv = psum_proj.tile([128, 512], f32, tag="proj")
                for c in range(EC):
                    nc.tensor.matmul(pkv[:], wkv_sb[:, c, :], hs_t[t][:, c, :],
                                     start=(c == 0), stop=(c == EC - 1))
                nc.vector.tensor_scalar_add(kv_t[t][:], pkv[:], bkv_sb[:])
                nc.sync.dma_start(kd_t[t][64:128, :], kv_t[t][0:64, :])
                if t < QB:
                    pq = psum_proj.tile([H, 512], f32, tag="proj")
                    for c in range(EC):
                        nc.tensor.matmul(pq[:], wq_sb[:, c, :], hs_t[t][:, c, :],
                                         start=(c == 0), stop=(c == EC - 1))
                    nc.vector.tensor_scalar_add(q_t[t][:], pq[:], bq_sb[:])
                    nc.sync.dma_start(qd_t[t][64:128, :], q_t[t][:])
                for kk in range(4):
                    kc = 4 * t + kk
                    ptr = psum_proj.tile([128, H], bf16, tag="proj")
                    nc.tensor.transpose(ptr[:], kv_t[t][64:128, bass.ts(kk, 128)],
                                        ident_bf[64:128, 64:128])
                    nc.vector.tensor_copy(vaug[kc][:, 0:H], ptr[:])

            # main loop: scores^T -> exp -> AV accumulate
            for qb in range(QB):
                pav = psum_av.tile([H + 1, 512], f32, tag="av")
                for g in range(KC // G):
                    pst = psum_st.tile([128, G, 512], f32, tag="st")
                    for i in range(G):
                        kc = G * g + i
                        if i % 2 == 0:
                            nc.tensor.matmul(pst[:, i, :],
                                             kv_t[kc // 4][0:64, bass.ts(kc % 4, 128)],
                                             q_t[qb][:],
                                             start=True, stop=True)
                        else:
                            nc.tensor.matmul(pst[:, i, :],
                                             kd_t[kc // 4][64:128, bass.ts(kc % 4, 128)],
                                             qd_t[qb][64:128, :],
                                             start=True, stop=True)
                    ptile = pt_pool.tile([128, G, 512], bf16, tag="pt")
                    nc.scalar.activation(ptile[:], pst[:],
                                         mybir.ActivationFunctionType.Exp)
                    for i in range(G):
                        kc = G * g + i
                        nc.tensor.matmul(pav[:], vaug[kc][:], ptile[:, i, :],
                                         start=(kc == 0), stop=(kc == KC - 1))
                # epilogue: normalize + transpose + store
                oT = oT_pool.tile([H + 1, 512], f32, tag="oT")
                nc.vector.tensor_copy(oT[:], pav[:])
                for j in range(4):
                    ptr2 = psum_proj.tile([128, H + 1], f32, tag="proj")
                    nc.tensor.transpose(ptr2[:], oT[:, bass.ts(j, 128)],
                                        ident_f32[0:H + 1, 0:H + 1])
                    rec = out_pool.tile([128, 1], f32, tag="rec")
                    nc.vector.reciprocal(rec[:], ptr2[:, H:H + 1])
                    osb = out_pool.tile([128, H], f32, tag="osb")
                    nc.vector.tensor_scalar_mul(osb[:], ptr2[:, 0:H], rec[:])
                    nc.sync.dma_start(
                        out.ap()[512 * qb + 128 * j:512 * qb + 128 * (j + 1), :],
                        osb[:])

    nc.compile()
    return nc


def _get_nc():
    global _nc_cache
    if _nc_cache is None:
        _nc_cache = _build()
    return _nc_cache


def kernel(hidden_state, Wq, bq, Wk, bk, Wv, bv):
    global last_results
    nc = _get_nc()
    scale = 1.0 / np.sqrt(np.float32(H))
    # weight layouts match the SBUF tiles: w[p, c, h] = W[c*128+p, h]
    wkv_np = np.ascontiguousarray(
        np.concatenate([Wk, Wv], axis=1).reshape(EC, 128, 128).transpose(1, 0, 2)
    ).astype(ml_dtypes.bfloat16)
    wq_np = np.ascontiguousarray(
        (Wq * scale).reshape(EC, 128, H).transpose(1, 0, 2)).astype(ml_dtypes.bfloat16)
    bkv_np = np.ascontiguousarray(
        np.concatenate([bk, bv]).reshape(128, 1)).astype(np.float32)
    bq_np = np.ascontiguousarray((bq * scale).reshape(H, 1)).astype(np.float32)

    in_maps = []
    for core in range(NCORES):
        b, qh = core // 2, core % 2
        hs = np.asarray(hidden_state[b], dtype=np.float32)
        if qh:
            hs = np.concatenate([hs[M:], hs[:M]], axis=0)
        # hsb[j, p, c, s] = hs.T[c*128+p, 512j+s]
        hsb_np = np.ascontiguousarray(
            hs.T.reshape(EC, 128, ST, 512).transpose(2, 1, 0, 3)
        ).astype(ml_dtypes.bfloat16)
        in_maps.append({"hsb": hsb_np, "wkv": wkv_np, "wq": wq_np,
                        "bkv": bkv_np, "bq": bq_np})

    last_results = run_bass_kernel_spmd(nc, in_maps, core_ids=list(range(NCORES)))
    out = np.empty((B, S, H), np.float32)
    for core in range(NCORES):
        b, qh = core // 2, core % 2
        out[b, M * qh:M * (qh + 1)] = last_results.results[core]["out"]
    return out


# revision 13
# speedup vs baseline: 1.1675x; 1.0063x over previous
"""Single-head attention on 8 TRN2 NeuronCores.

Sharding: data-parallel over batch (4) x sequence-parallel over the query
dim (2 halves of 2048) = 8 independent shards, no collectives. Each core
receives its batch's hidden_state pre-transposed ([E, S] bf16) with the
local query block rotated to columns 0:2048 (key order is irrelevant to
softmax+AV as long as K and V share it).

Per-core kernel (all matmuls bf16, PSUM f32):
  hsT [1024, 4096]
  KV^T = [Wk|Wv]^T @ hsT      -> kvT [128, 4096]  (rows 0:64 K^T, 64:128 V^T)
  Q^T  = (Wq*s)^T @ hsT[:, :2048] -> qT [64, 2048]   (s = 1/sqrt(64) folded in)
  V    = transpose(V^T) chunks -> vaug [128, 65] per k-chunk (col 64 = ones)
  S^T[k,q] = K^T.T @ Q^T  (per 128-k-chunk, 512-q-block)
  P^T = exp(S^T)            (scores bounded ~|2|, no max-subtraction needed)
  outT[65, q] = sum_k vaug^T @ P^T   (row 64 = softmax denominator)
  out[q, 64] = transpose(outT) * recip(denominator)
"""

import numpy as np
import ml_dtypes
from contextlib import ExitStack

import concourse.bass as bass
import concourse.bacc as bacc
import concourse.mybir as mybir
import concourse.tile as tile
from concourse.bass_utils import run_bass_kernel_spmd
from concourse.masks import make_identity

B, S, E, H = 4, 4096, 1024, 64
M = 2048            # queries per core
NCORES = 8
EC = E // 128       # 8 contraction chunks
ST = S // 512       # 8 projection column tiles
KC = S // 128       # 32 key chunks
QB = M // 512       # 4 query blocks
G = 2               # key chunks per exp group (psum banks per S^T tile)
bf16 = mybir.dt.bfloat16
f32 = mybir.dt.float32

_nc_cache = None
last_results = None  # BassKernelResults from the most recent run (for test.py)


def _build():
    nc = bacc.Bacc("TRN2", target_bir_lowering=False, debug=False,
                   num_devices=NCORES)
    # hsb[j, p, c, s] = hsT[c*128+p, 512j+s] — every DMA fully contiguous
    hsb = nc.dram_tensor("hsb", [ST, 128, EC, 512], bf16, kind="ExternalInput")
    wkv = nc.dram_tensor("wkv", [128, EC, 128], bf16, kind="ExternalInput")
    wq = nc.dram_tensor("wq", [128, EC, H], bf16, kind="ExternalInput")
    bkv = nc.dram_tensor("bkv", [128, 1], f32, kind="ExternalInput")
    bqt = nc.dram_tensor("bq", [H, 1], f32, kind="ExternalInput")
    out = nc.dram_tensor("out", [M, H], f32, kind="ExternalOutput")

    with tile.TileContext(nc) as tc:
        with ExitStack() as ctx:
            consts = ctx.enter_context(tc.tile_pool(name="consts", bufs=1))
            sb = ctx.enter_context(tc.tile_pool(name="sb", bufs=1))
            pt_pool = ctx.enter_context(tc.tile_pool(name="pt", bufs=3))
            oT_pool = ctx.enter_context(tc.tile_pool(name="oT", bufs=2))
            out_pool = ctx.enter_context(tc.tile_pool(name="osb", bufs=3))
            psum_proj = ctx.enter_context(
                tc.tile_pool(name="pp", bufs=2, space=bass.MemorySpace.PSUM))
            psum_st = ctx.enter_context(
                tc.tile_pool(name="pst", bufs=2, space=bass.MemorySpace.PSUM))
            psum_av = ctx.enter_context(
                tc.tile_pool(name="pav", bufs=2, space=bass.MemorySpace.PSUM))

            wkv_sb = consts.tile([128, EC, 128], bf16)
            nc.sync.dma_start(wkv_sb[:], wkv.ap())
            wq_sb = consts.tile([128, EC, H], bf16)
            nc.sync.dma_start(wq_sb[:], wq.ap())
            bkv_sb = consts.tile([128, 1], f32)
            nc.sync.dma_start(bkv_sb[:], bkv.ap())
            bq_sb = consts.tile([H, 1], f32)
            nc.sync.dma_start(bq_sb[:], bqt.ap())
            ident_bf = consts.tile([128, 128], bf16)
            make_identity(nc, ident_bf[:])
            ident_f32 = consts.tile([128, 128], f32)
            make_identity(nc, ident_f32[:])

            # input: one tile per 512-column block; two DMAs per block so the
            # first projection chunks land sooner and spread more queues
            hs_t = []
            for j in range(ST):
                t = sb.tile([128, EC, 512], bf16, tag=f"hs{j}", name=f"hs{j}")
                nc.sync.dma_start(t[:, 0:EC // 2, :], hsb.ap()[j, :, 0:EC // 2, :])
                nc.sync.dma_start(t[:, EC // 2:, :], hsb.ap()[j, :, EC // 2:, :])
                hs_t.append(t)

            kv_t = [sb.tile([128, 512], bf16, tag=f"kv{t}", name=f"kv{t}") for t in range(ST)]
            q_t = [sb.tile([H, 512], bf16, tag=f"q{t}", name=f"q{t}") for t in range(QB)]
            # copies of K^T / Q^T at partition offset 64 so score matmul
            # pairs can row-tile the PE array (rows 0:63 and 64:127 run
            # concurrently: the contraction dim is only h=64)
            kd_t = [sb.tile([128, 512], bf16, tag=f"kd{t}", name=f"kd{t}") for t in range(ST)]
            qd_t = [sb.tile([128, 512], bf16, tag=f"qd{t}", name=f"qd{t}") for t in range(QB)]
            vaug = [sb.tile([128, H + 1], bf16, tag=f"v{kc}", name=f"v{kc}") for kc in range(KC)]
            for kc in range(KC):
                nc.gpsimd.memset(vaug[kc][:, H:H + 1], 1.0)

            pav_of = {}

            def emit_group(qb, g):
                pst = psum_st.tile([128, G, 512], f32, tag="st",
                                   name=f"pst{qb}_{g}")
                for i in range(G):
                    kc = G * g + i
                    if i % 2 == 0:
                        nc.tensor.matmul(pst[:, i, :],
                                         kv_t[kc // 4][0:64, bass.ts(kc % 4, 128)],
                                         q_t[qb][:],
                                         start=True, stop=True)
                    else:
                        nc.tensor.matmul(pst[:, i, :],
                                         kd_t[kc // 4][64:128, bass.ts(kc % 4, 128)],
                                         qd_t[qb][64:128, :],
                                         start=True, stop=True)
                ptile = pt_pool.tile([128, G, 512], bf16, tag="pt",
                                     name=f"pt{qb}_{g}")
                nc.scalar.activation(ptile[:], pst[:],
                                     mybir.ActivationFunctionType.Exp)
                for i in range(G):
                    kc = G * g + i
                    nc.tensor.matmul(pav_of[qb][:], vaug[kc][:], ptile[:, i, :],
                                     start=(kc == 0), stop=(kc == KC - 1))

            def emit_epilogue(qb):
                oT = oT_pool.tile([H + 1, 512], f32, tag="oT", name=f"oT{qb}")
                nc.vector.tensor_copy(oT[:], pav_of[qb][:])
                for j in range(4):
                    ptr2 = psum_proj.tile([128, H + 1], f32, tag="proj",
                                          name=f"otr{qb}_{j}")
                    nc.tensor.transpose(ptr2[:], oT[:, bass.ts(j, 128)],
                                        ident_f32[0:H + 1, 0:H + 1])
                    rec = out_pool.tile([128, 1], f32, tag="rec",
                                        name=f"rec{qb}_{j}")
                    nc.vector.reciprocal(rec[:], ptr2[:, H:H + 1])
                    osb = out_pool.tile([128, H], f32, tag="osb",
                                        name=f"osb{qb}_{j}")
                    nc.vector.tensor_scalar_mul(osb[:], ptr2[:, 0:H], rec[:])
                    nc.sync.dma_start(
                        out.ap()[512 * qb + 128 * j:512 * qb + 128 * (j + 1), :],
                        osb[:])

            # projections + V transposes, with q-blocks 0/1's score groups
            # emitted as soon as their k-chunks are projected so the exp
            # pipeline starts during the DMA/projection phase
            emitted = {0: 0, 1: 0}
            for t in range(ST):
                pkv = psum_proj.tile([128, 512], f32, tag="proj",
                                     name=f"pkv{t}")
                for c in range(EC):
                    nc.tensor.matmul(pkv[:], wkv_sb[:, c, :], hs_t[t][:, c, :],
                                     start=(c == 0), stop=(c == EC - 1))
                nc.vector.tensor_scalar_add(kv_t[t][:], pkv[:], bkv_sb[:])
                nc.sync.dma_start(kd_t[t][64:128, :], kv_t[t][0:64, :])
                if t < QB:
                    pq = psum_proj.tile([H, 512], f32, tag="proj",
                                        name=f"pq{t}")
                    for c in range(EC):
                        nc.tensor.matmul(pq[:], wq_sb[:, c, :], hs_t[t][:, c, :],
                                         start=(c == 0), stop=(c == EC - 1))
                    nc.vector.tensor_scalar_add(q_t[t][:], pq[:], bq_sb[:])
                    nc.sync.dma_start(qd_t[t][64:128, :], q_t[t][:])
                for kk in range(4):
                    kc = 4 * t + kk
                    ptr = psum_proj.tile([128, H], bf16, tag="proj",
                                         name=f"vtr{kc}")
                    nc.tensor.transpose(ptr[:], kv_t[t][64:128, bass.ts(kk, 128)],
                                        ident_bf[64:128, 64:128])
                    nc.vector.tensor_copy(vaug[kc][:, 0:H], ptr[:])
                # interleave ready score groups for qb 0 and 1
                ready_g = (4 * (t + 1)) // G
                for qb in (0, 1):
                    if t < qb:
                        continue
                    if qb not in pav_of:
                        pav_of[qb] = psum_av.tile([H + 1, 512], f32, tag="av",
                                                  name=f"pav{qb}")
                    while emitted[qb] < ready_g:
                        emit_group(qb, emitted[qb])
                        emitted[qb] += 1
            emit_epilogue(0)
            emit_epilogue(1)

            for qb in (2, 3):
                pav_of[qb] = psum_av.tile([H + 1, 512], f32, tag="av",
                                          name=f"pav{qb}")
                for g in range(KC // G):
                    emit_group(qb, g)
                emit_epilogue(qb)

    nc.compile()
    return nc


def _get_nc():
    global _nc_cache
    if _nc_cache is None:
        _nc_cache = _build()
    return _nc_cache


def kernel(hidden_state, Wq, bq, Wk, bk, Wv, bv):
    global last_results
    nc = _get_nc()
    scale = 1.0 / np.sqrt(np.float32(H))
    # weight layouts match the SBUF tiles: w[p, c, h] = W[c*128+p, h]
    wkv_np = np.ascontiguousarray(
        np.concatenate([Wk, Wv], axis=1).reshape(EC, 128, 128).transpose(1, 0, 2)
    ).astype(ml_dtypes.bfloat16)
    wq_np = np.ascontiguousarray(
        (Wq * scale).reshape(EC, 128, H).transpose(1, 0, 2)).astype(ml_dtypes.bfloat16)
    bkv_np = np.ascontiguousarray(
        np.concatenate([bk, bv]).reshape(128, 1)).astype(np.float32)
    bq_np = np.ascontiguousarray((bq * scale).reshape(H, 1)).astype(np.float32)

    in_maps = []
    for core in range(NCORES):
        b, qh = core // 2, core % 2
        hs = np.asarray(hidden_state[b], dtype=np.float32)
        if qh:
            hs = np.concatenate([hs[M:], hs[:M]], axis=0)
        # hsb[j, p, c, s] = hs.T[c*128+p, 512j+s]
        hsb_np = np.ascontiguousarray(
            hs.T.reshape(EC, 128, ST, 512).transpose(2, 1, 0, 3)
        ).astype(ml_dtypes.bfloat16)
        in_maps.append({"hsb": hsb_np, "wkv": wkv_np, "wq": wq_np,
                        "bkv": bkv_np, "bq": bq_np})

    last_results = run_bass_kernel_spmd(nc, in_maps, core_ids=list(range(NCORES)))
    out = np.empty((B, S, H), np.float32)
    for core in range(NCORES):
        b, qh = core // 2, core % 2
        out[b, M * qh:M * (qh + 1)] = last_results.results[core]["out"]
    return out
